# revision 32
# baseline (speedup 1.0000x reference)
"""Trainium2 Bass kernel for nn_CapsuleLayer (dynamic-routing capsule layer).

Reference computation:
    priors = einsum('bni,cnio->cbno', x, W)            # [32c, 64b, 1152n, 32o]
    3 iterations of routing over n (capsules):
        probs = softmax(logits, axis=n)
        s = sum_n(probs * priors);  outputs = squash(s)
        logits += sum_o(priors * outputs)
    return outputs  # [32, 64, 1, 1, 32]

Sharding: 8 cores x 4 classes each (routing is independent per (class,batch));
each core reads only its 4-class slice of W (18.9MB) plus x -- W traffic is
the unavoidable HBM floor, so never replicate it.

v2 structure (per core):
  - Priors via n-PAIR matmuls: host packs x block-diagonally so one matmul
    covers two capsules: lhsT = xpad_t [K=64=(g,i), M=128=(g,b)] (block-diag,
    assembled on-chip from a compact x DMA into a pre-zeroed tile), rhs =
    W_t [64, 64=(cc,o)] -> psum [128=(par,b), slot, (cc,o)].  576 matmuls per
    class-pair instead of 1152 (the baseline was bound by LDWEIGHTS issue
    overhead at 220ns each).
  - it-0 s-accumulation is folded into the priors phase: per psum tile a DVE
    tensor_reduce over the 16 slots accumulates s0 partials, so routing
    starts at squash(s0) instead of re-reading 9.4MB of priors.
  - Drains psum->L (o-major SBUF layout) run on the Scalar engine, keeping
    DVE free for routing math.
  - Routing per class entirely without broadcasts, using o-slice ops:
      s-pass:  scalar_tensor_tensor(out=p_o*e, accum_out=sum_nr) per o (DVE)
      delta:   STT chains on DVE + tensor_scalar/tensor_add pairs on GpSimd
    Parity-fold + broadcast-to-128-partitions via one tiny matmul with the
    constant selector sel2[128,128] = tile(I_64).  exp on ScalarE with
    accum_out gives the softmax normalizer Z for free; squash factor
    sqrt(sq)/(1+sq) via Ln/Exp + vector reciprocal.

All fp32: bf16 anywhere in the priors/logits path loses 10-23% accuracy
(routing softmax has logits up to +-80 and amplifies error exponentially).
"""

from contextlib import ExitStack

import numpy as np

import concourse.bass as bass
import concourse.mybir as mybir
import concourse.tile as tile
from concourse import bass_utils

AF = mybir.ActivationFunctionType
ALU = mybir.AluOpType
FP32 = mybir.dt.float32

C, B, N, I, O = 32, 64, 1152, 32, 32
N_CORES = 8
CLASSES_PER_CORE = C // N_CORES          # 4
PAIRS = CLASSES_PER_CORE // 2            # 2 class-pairs per core
NR = N // 2                              # 576 n-pairs (n = 2*nr + parity)
T = N // 4                               # 288 4-n host blocks
INV_N = 1.0 / N

USE_FP32R = False                        # fp32r: imprecise (3.2e-2) AND slow
USE_FP16 = False                         # fp16 matmul: fast but 2.3e-2 > gate
T_CHUNK = 4                              # 16 n per DMA chunk
SLOTS = 16                               # nr slots per psum drain (32 n)
# delta o-slice split: DVE STT-slice ~800ns vs GpSimd 2-op TT ~3.2us
DVE_O = 21                               # o-slices on DVE; rest on GpSimd


def _legalize_syncs(nc):
    """This container's walrus codegen allows at most ONE sync-wait command
    per instruction, and a `sem-eq-imm` wait encodes as two commands.  Tile's
    tail barrier emits follower drains with `wait release==0` (vacuous at the
    kernel tail) and matmuls can accumulate 2+ DMA waits.  Fix both by
    post-processing the scheduled IR: strip the vacuous eq-0 drain waits and
    hoist all-but-one ge-waits onto injected same-engine NoOps."""
    nid = 0
    for f in nc.m.functions:
        for blk in f.blocks:
            newlist = []
            changed = False
            for ins in blk.instructions:
                si = ins.sync_info
                if si is None or not si.on_wait:
                    newlist.append(ins)
                    continue
                waits = list(si.on_wait)
                if isinstance(ins, mybir.InstDrain):
                    kept = [w for w in waits
                            if not (w.wait_mode == "sem-eq-imm"
                                    and w.wait_value == 0)]
                    if len(kept) != len(waits):
                        changed = True
                    waits = kept
                while len(waits) > 1:
                    w = waits.pop(0)
                    ev = mybir.InstEventSemaphore(
                        name=f"syncfix_{nid}", ins=[], outs=[])
                    nid += 1
                    ev.engine = ins.engine
                    ev.sync_info = mybir.SyncInfo(on_wait=[w], on_update=[])
                    newlist.append(ev)
                    changed = True
                ins.sync_info = mybir.SyncInfo(on_wait=waits,
                                               on_update=list(si.on_update))
                newlist.append(ins)
            if changed:
                blk.instructions = newlist
    return nc


def _build_program(nc: bass.Bass, legalize: bool = True):
    if USE_FP16:
        mm_dt = mybir.dt.float16
    elif USE_FP32R:
        mm_dt = mybir.dt.float32r
    else:
        mm_dt = FP32
    # xT[t, i, g, b] = x[b, 4t+g, i];  Wr[pair, t, i, g, (cc,o)] interleaved.
    xT = nc.dram_tensor("xT", [T, I, 4, B], mm_dt, kind="ExternalInput").ap()
    Wr = nc.dram_tensor("Wr", [PAIRS, T, I, 4, 64], mm_dt,
                        kind="ExternalInput").ap()
    sel2 = nc.dram_tensor("sel2", [128, 128], FP32, kind="ExternalInput").ap()
    out4 = nc.dram_tensor("out4", [CLASSES_PER_CORE, B, O], FP32,
                          kind="ExternalOutput").ap()

    with tile.TileContext(nc) as tc, ExitStack() as ctx:  # noqa: SIM117
        pools = {
            "consts": ctx.enter_context(tc.tile_pool(name="consts", bufs=1)),
            "xw": ctx.enter_context(tc.tile_pool(name="xw", bufs=3)),
            "lpool": ctx.enter_context(tc.tile_pool(name="lpool", bufs=1)),
            "pr_psum": ctx.enter_context(
                tc.tile_pool(name="pr_psum", bufs=2, space="PSUM")),
            "fold_psum": ctx.enter_context(
                tc.tile_pool(name="fold_psum", bufs=2, space="PSUM")),
            "scratch": ctx.enter_context(tc.tile_pool(name="scratch", bufs=1)),
            "small": ctx.enter_context(tc.tile_pool(name="small", bufs=4)),
        }
        nc_ = tc.nc
        sel_t = pools["consts"].tile([128, 128], FP32)
        nc_.sync.dma_start(out=sel_t[:], in_=sel2)

        for pair in range(PAIRS):
            s0p = pools["small"].tile([128, 2, O], FP32, tag=f"s0p{pair}",
                                      name=f"s0p{pair}")
            Ls = [pools["lpool"].tile([128, O, NR], FP32, tag=f"L{cc}",
                                      name=f"L{pair}_{cc}") for cc in range(2)]
            _priors_phase(tc, xT, Wr, pair, Ls, s0p, pools)
            # interleave the two classes' emission at step granularity so the
            # in-order engine queues never head-of-line block on one class's
            # dependency chain while the other class has ready work.
            gens = [_route_class(tc, Ls[cc], sel_t, cc, pair, s0p, out4,
                                 pools) for cc in range(2)]
            live = list(gens)
            while live:
                for g in list(live):
                    try:
                        next(g)
                    except StopIteration:
                        live.remove(g)
    if legalize:
        _legalize_syncs(nc)
    return nc


def _priors_phase(tc, xT, Wr, pair, Ls, s0p, pools):
    nc = tc.nc
    mm_dt = xT.dtype
    # one psum drain per 32 n = 8 t-blocks = 2 DMA chunks
    for drain in range(N // 32):
        psum_t = pools["pr_psum"].tile([128, SLOTS, 64], FP32, tag="prpsum")
        for sub in range(2):
            chunk = drain * 2 + sub
            t0 = chunk * T_CHUNK
            x_tile = pools["xw"].tile([I, T_CHUNK, 4, B], mm_dt, tag="x")
            w_tile = pools["xw"].tile([I, T_CHUNK, 4, 64], mm_dt, tag="w")
            nc.sync.dma_start(
                out=x_tile[:],
                in_=xT[t0:t0 + T_CHUNK].rearrange("t p g b -> p t g b"))
            nc.sync.dma_start(
                out=w_tile[:],
                in_=Wr[pair, t0:t0 + T_CHUNK].rearrange("t p g f -> p t g f"))
            for tt in range(T_CHUNK):
                for g in range(4):
                    n_local = (sub * T_CHUNK + tt) * 4 + g
                    slot = (n_local // 2) % SLOTS
                    par = g & 1            # n parity = g parity (n = 4t+g)
                    nc.tensor.matmul(
                        psum_t[par * 64:(par + 1) * 64, slot, :],
                        x_tile[:, tt, g, :],
                        w_tile[:, tt, g, :],
                        start=True, stop=True,
                        tile_position=(0, par * 64))
        # drains on ScalarE (o-major into per-class L), it0 partial on DVE
        for cc in range(2):
            lview = Ls[cc][:, :, drain * SLOTS:(drain + 1) * SLOTS]
            nc.scalar.copy(lview.rearrange("p o s -> p s o"),
                           psum_t[:, :, cc * O:(cc + 1) * O])
        part = pools["small"].tile([128, 2, O], FP32, tag="s0part")
        nc.vector.tensor_reduce(
            part[:], psum_t[:].rearrange("p s (c o) -> p c o s", c=2),
            axis=mybir.AxisListType.X, op=ALU.add)
        if drain == 0:
            nc.vector.tensor_copy(s0p[:], part[:])
        else:
            nc.vector.tensor_add(s0p[:], s0p[:], part[:])


def _route_class(tc, Lc, sel_t, cc, pair, s0p, out4, pools):
    nc = tc.nc
    scratch, small, fold_psum = pools["scratch"], pools["small"], pools["fold_psum"]
    ctag = f"c{cc}"

    junk = scratch.tile([128, NR], FP32, tag=f"junk{ctag}")
    e_t = scratch.tile([128, NR], FP32, tag=f"e{ctag}")
    lG = scratch.tile([128, NR], FP32, tag=f"lG{ctag}")
    gtmp = scratch.tile([128, NR], FP32, tag=f"gtmp{ctag}")
    dbufs = [scratch.tile([128, NR], FP32, tag=f"ld{i}{ctag}",
                          name=f"ld{i}{ctag}{pair}")
             for i in range(2)]
    # sp holds the 32 weighted o-sums plus the softmax normalizer Z in col 32
    sp = small.tile([128, O + 1], FP32, tag=f"sp{ctag}")

    def fold(src_ap, ncols, tag, scale=None):
        """[128=(par,b), ncols] -> [128, ncols] (parity-summed, b-replicated
        across both partition halves) via sel2 matmul; lands in SBUF."""
        ps = fold_psum.tile([128, 64], FP32, tag="foldps")
        nc.tensor.matmul(ps[:, :ncols], sel_t[:], src_ap,
                         start=True, stop=True)
        dst = small.tile([128, ncols], FP32, tag=tag + ctag)
        if scale is None:
            nc.scalar.copy(dst[:], ps[:, :ncols])
        else:
            nc.scalar.activation(dst[:], ps[:, :ncols], AF.Copy, scale=scale)
        return dst

    def s_accumulate(weight):
        """sp[:, o] = sum_nr Lc[:, o, :] * weight[:, :]  (all DVE)."""
        for o in range(O):
            nc.vector.scalar_tensor_tensor(
                out=junk[:], in0=Lc[:, o, :], scalar=0.0, in1=weight[:],
                op0=ALU.bypass, op1=ALU.mult, accum_out=sp[:, o:o + 1])

    def delta_accumulate(outs, prev):
        """Return tile holding (prev-logits or 0) + sum_o Lc[:,o,:]*outs[:,o].
        DVE chains o<DVE_O (seeded with prev), GpSimd chains the rest."""
        cur = prev
        for o in range(DVE_O):
            dst = dbufs[0] if cur is not dbufs[0] else dbufs[1]
            if cur is None:
                nc.vector.scalar_tensor_tensor(
                    out=dst[:], in0=Lc[:, o, :], scalar=outs[:, o:o + 1],
                    in1=Lc[:, o, :], op0=ALU.mult, op1=ALU.bypass)
            else:
                nc.vector.scalar_tensor_tensor(
                    out=dst[:], in0=Lc[:, o, :], scalar=outs[:, o:o + 1],
                    in1=cur[:], op0=ALU.mult, op1=ALU.add)
            cur = dst
        first = True
        for o in range(DVE_O, O):
            # gpsimd tensor_scalar w/ AP scalar measured 8.7us -- use the
            # broadcast multiply (1.7us) then accumulate-add instead.
            if first:
                nc.gpsimd.tensor_mul(lG[:], Lc[:, o, :],
                                     outs[:, o:o + 1].to_broadcast((128, NR)))
                first = False
            else:
                nc.gpsimd.tensor_mul(gtmp[:], Lc[:, o, :],
                                     outs[:, o:o + 1].to_broadcast((128, NR)))
                nc.gpsimd.tensor_add(lG[:], lG[:], gtmp[:])
        nc.vector.tensor_add(cur[:], cur[:], lG[:])
        return cur

    def squash(s_sb):
        """outputs = s * sqrt(sq)/(1+sq), all [128, O] per-partition."""
        sq = small.tile([128, 1], FP32, tag="sq" + ctag)
        nc.vector.scalar_tensor_tensor(
            out=junk[:, :O], in0=s_sb[:], scalar=0.0, in1=s_sb[:],
            op0=ALU.bypass, op1=ALU.mult, accum_out=sq[:])
        lnq = small.tile([128, 1], FP32, tag="lnq" + ctag)
        nc.scalar.activation(lnq[:], sq[:], AF.Ln)
        r = small.tile([128, 1], FP32, tag="r" + ctag)
        nc.scalar.activation(r[:], lnq[:], AF.Exp, scale=0.5)  # sqrt(sq)
        q1 = small.tile([128, 1], FP32, tag="q1" + ctag)
        nc.vector.tensor_scalar_add(q1[:], sq[:], 1.0)
        iq = small.tile([128, 1], FP32, tag="iq" + ctag)
        nc.vector.reciprocal(iq[:], q1[:])
        f = small.tile([128, 1], FP32, tag="f" + ctag)
        nc.vector.tensor_mul(f[:], r[:], iq[:])
        outs = small.tile([128, O], FP32, tag="outs" + ctag)
        nc.vector.tensor_scalar(out=outs[:], in0=s_sb[:], scalar1=f[:],
                                scalar2=None, op0=ALU.mult)
        return outs

    # iteration 0: s0 = mean_n p (already accumulated from psum partials)
    s_sb = fold(s0p[:, cc], O, "s_sb", scale=INV_N)
    outs = squash(s_sb)
    yield
    logits = delta_accumulate(outs, None)
    yield

    for it in (1, 2):
        nc.scalar.activation(e_t[:], logits[:], AF.Exp,
                             accum_out=sp[:, O:O + 1])
        s_accumulate(e_t)
        yield
        sraw = fold(sp[:], O + 1, "s_sb")       # cols 0..31 = s, col 32 = Z
        iz = small.tile([128, 1], FP32, tag="iz" + ctag)
        nc.vector.reciprocal(iz[:], sraw[:, O:O + 1])
        s_sb = small.tile([128, O], FP32, tag="s_n" + ctag)
        nc.vector.tensor_scalar(out=s_sb[:], in0=sraw[:, :O], scalar1=iz[:],
                                scalar2=None, op0=ALU.mult)
        outs = squash(s_sb)
        yield
        if it == 1:
            logits = delta_accumulate(outs, logits)
            yield
        else:
            nc.sync.dma_start(out=out4[pair * 2 + cc], in_=outs[:B, :])


# ---------------------------------------------------------------------------
# host-side entry point
# ---------------------------------------------------------------------------

_COMPILED = {}


def _prep_host_inputs(x, route_weights):
    mm_np = np.float16 if USE_FP16 else np.float32
    x = np.ascontiguousarray(x, dtype=np.float32)
    W = np.ascontiguousarray(route_weights, dtype=np.float32)
    # xT[t, i, g, b] = x[b, 4t+g, i]
    xT = np.ascontiguousarray(
        x.reshape(B, T, 4, I).transpose(1, 3, 2, 0).astype(mm_np))
    sel2 = np.tile(np.eye(B, dtype=np.float32), (2, 2)).astype(np.float32)
    sel2 = np.ascontiguousarray(sel2)

    in_maps = []
    for k in range(N_CORES):
        Wk = W[k * CLASSES_PER_CORE:(k + 1) * CLASSES_PER_CORE]
        # Wr[pair, t, i, g, (cc,o)] = Wk[2*pair+cc, 4t+g, i, o]
        Wr = Wk.reshape(PAIRS, 2, T, 4, I, O).transpose(0, 2, 4, 3, 1, 5)
        Wr = np.ascontiguousarray(
            Wr.reshape(PAIRS, T, I, 4, 64).astype(mm_np))
        in_maps.append({"xT": xT, "Wr": Wr, "sel2": sel2})
    return in_maps


def _get_compiled():
    if "nc" not in _COMPILED:
        nc = bass.Bass("TRN2", target_bir_lowering=False, debug=False,
                       enable_asserts=False, num_devices=N_CORES)
        _build_program(nc)
        _COMPILED["nc"] = nc
    return _COMPILED["nc"]


def kernel(x, route_weights, **run_kwargs):
    in_maps = _prep_host_inputs(x, route_weights)
    nc = _get_compiled()
    res = bass_utils.run_bass_kernel_spmd(
        nc, in_maps, core_ids=list(range(N_CORES)), **run_kwargs)
    full = np.concatenate([r["out4"] for r in res.results], axis=0)
    out = full[:, :, None, None, :].astype(np.float32)
    if run_kwargs:
        kernel.last_results = res
    return out


if __name__ == "__main__":
    rng = np.random.default_rng(0)
    xs = rng.standard_normal((B, N, I), dtype=np.float32)
    ws = rng.standard_normal((C, N, I, O), dtype=np.float32)
    print(kernel(xs, ws).shape)


# revision 33
# speedup vs baseline: 1.2870x; 1.2870x over previous
"""Trainium2 Bass kernel for nn_CapsuleLayer (dynamic-routing capsule layer).

Reference computation:
    priors = einsum('bni,cnio->cbno', x, W)            # [32c, 64b, 1152n, 32o]
    3 iterations of routing over n (capsules):
        probs = softmax(logits, axis=n)
        s = sum_n(probs * priors);  outputs = squash(s)
        logits += sum_o(priors * outputs)
    return outputs  # [32, 64, 1, 1, 32]

Sharding: 8 cores x 4 classes each (routing is independent per (class,batch));
each core reads only its 4-class slice of W (18.9MB) plus x -- W traffic is
the unavoidable HBM floor, so never replicate it.

v2 structure (per core):
  - Priors via n-PAIR matmuls: host packs x block-diagonally so one matmul
    covers two capsules: lhsT = xpad_t [K=64=(g,i), M=128=(g,b)] (block-diag,
    assembled on-chip from a compact x DMA into a pre-zeroed tile), rhs =
    W_t [64, 64=(cc,o)] -> psum [128=(par,b), slot, (cc,o)].  576 matmuls per
    class-pair instead of 1152 (the baseline was bound by LDWEIGHTS issue
    overhead at 220ns each).
  - it-0 s-accumulation is folded into the priors phase: per psum tile a DVE
    tensor_reduce over the 16 slots accumulates s0 partials, so routing
    starts at squash(s0) instead of re-reading 9.4MB of priors.
  - Drains psum->L (o-major SBUF layout) run on the Scalar engine, keeping
    DVE free for routing math.
  - Routing per class entirely without broadcasts, using o-slice ops:
      s-pass:  scalar_tensor_tensor(out=p_o*e, accum_out=sum_nr) per o (DVE)
      delta:   STT chains on DVE + tensor_scalar/tensor_add pairs on GpSimd
    Parity-fold + broadcast-to-128-partitions via one tiny matmul with the
    constant selector sel2[128,128] = tile(I_64).  exp on ScalarE with
    accum_out gives the softmax normalizer Z for free; squash factor
    sqrt(sq)/(1+sq) via Ln/Exp + vector reciprocal.

All fp32: bf16 anywhere in the priors/logits path loses 10-23% accuracy
(routing softmax has logits up to +-80 and amplifies error exponentially).
"""

from contextlib import ExitStack

import numpy as np

import concourse.bass as bass
import concourse.mybir as mybir
import concourse.tile as tile
from concourse import bass_utils

AF = mybir.ActivationFunctionType
ALU = mybir.AluOpType
FP32 = mybir.dt.float32

C, B, N, I, O = 32, 64, 1152, 32, 32
N_CORES = 8
CLASSES_PER_CORE = C // N_CORES          # 4
PAIRS = CLASSES_PER_CORE // 2            # 2 class-pairs per core
NR = N // 2                              # 576 n-pairs (n = 2*nr + parity)
T = N // 4                               # 288 4-n host blocks
INV_N = 1.0 / N

USE_FP32R = False                        # fp32r: imprecise (3.2e-2) AND slow
USE_FP16 = False                         # fp16 matmul: fast but 2.3e-2 > gate
T_CHUNK = 4                              # 16 n per DMA chunk
SLOTS = 16                               # nr slots per psum drain (32 n)
# delta o-slice split: DVE STT-slice ~800ns vs GpSimd 2-op TT ~3.2us
DVE_O = 25                               # o-slices on DVE; rest on GpSimd


def _legalize_syncs(nc):
    """This container's walrus codegen allows at most ONE sync-wait command
    per instruction, and a `sem-eq-imm` wait encodes as two commands.  Tile's
    tail barrier emits follower drains with `wait release==0` (vacuous at the
    kernel tail) and matmuls can accumulate 2+ DMA waits.  Fix both by
    post-processing the scheduled IR: strip the vacuous eq-0 drain waits and
    hoist all-but-one ge-waits onto injected same-engine NoOps."""
    nid = 0
    for f in nc.m.functions:
        for blk in f.blocks:
            newlist = []
            changed = False
            for ins in blk.instructions:
                si = ins.sync_info
                if si is None or not si.on_wait:
                    newlist.append(ins)
                    continue
                waits = list(si.on_wait)
                if isinstance(ins, mybir.InstDrain):
                    kept = [w for w in waits
                            if not (w.wait_mode == "sem-eq-imm"
                                    and w.wait_value == 0)]
                    if len(kept) != len(waits):
                        changed = True
                    waits = kept
                while len(waits) > 1:
                    w = waits.pop(0)
                    ev = mybir.InstEventSemaphore(
                        name=f"syncfix_{nid}", ins=[], outs=[])
                    nid += 1
                    ev.engine = ins.engine
                    ev.sync_info = mybir.SyncInfo(on_wait=[w], on_update=[])
                    newlist.append(ev)
                    changed = True
                ins.sync_info = mybir.SyncInfo(on_wait=waits,
                                               on_update=list(si.on_update))
                newlist.append(ins)
            if changed:
                blk.instructions = newlist
    return nc


def _build_program(nc: bass.Bass, legalize: bool = True):
    if USE_FP16:
        mm_dt = mybir.dt.float16
    elif USE_FP32R:
        mm_dt = mybir.dt.float32r
    else:
        mm_dt = FP32
    # xT[t, i, g, b] = x[b, 4t+g, i];  Wr[pair, t, i, g, (cc,o)] interleaved.
    xT = nc.dram_tensor("xT", [T, I, 4, B], mm_dt, kind="ExternalInput").ap()
    Wr = nc.dram_tensor("Wr", [PAIRS, T, I, 4, 64], mm_dt,
                        kind="ExternalInput").ap()
    sel2 = nc.dram_tensor("sel2", [128, 128], FP32, kind="ExternalInput").ap()
    out4 = nc.dram_tensor("out4", [CLASSES_PER_CORE, B, O], FP32,
                          kind="ExternalOutput").ap()

    with tile.TileContext(nc) as tc, ExitStack() as ctx:  # noqa: SIM117
        pools = {
            "consts": ctx.enter_context(tc.tile_pool(name="consts", bufs=1)),
            "xw": ctx.enter_context(tc.tile_pool(name="xw", bufs=3)),
            "lpool": ctx.enter_context(tc.tile_pool(name="lpool", bufs=1)),
            "pr_psum": ctx.enter_context(
                tc.tile_pool(name="pr_psum", bufs=2, space="PSUM")),
            "fold_psum": ctx.enter_context(
                tc.tile_pool(name="fold_psum", bufs=2, space="PSUM")),
            "scratch": ctx.enter_context(tc.tile_pool(name="scratch", bufs=1)),
            "small": ctx.enter_context(tc.tile_pool(name="small", bufs=4)),
        }
        nc_ = tc.nc
        sel_t = pools["consts"].tile([128, 128], FP32)
        nc_.sync.dma_start(out=sel_t[:], in_=sel2)

        for pair in range(PAIRS):
            s0p = pools["small"].tile([128, 2, O], FP32, tag=f"s0p{pair}",
                                      name=f"s0p{pair}")
            Ls = [pools["lpool"].tile([128, O, NR], FP32, tag=f"L{cc}",
                                      name=f"L{pair}_{cc}") for cc in range(2)]
            _priors_phase(tc, xT, Wr, pair, Ls, s0p, pools)
            # interleave the two classes' emission at step granularity so the
            # in-order engine queues never head-of-line block on one class's
            # dependency chain while the other class has ready work.
            gens = [_route_class(tc, Ls[cc], sel_t, cc, pair, s0p, out4,
                                 pools) for cc in range(2)]
            live = list(gens)
            while live:
                for g in list(live):
                    try:
                        next(g)
                    except StopIteration:
                        live.remove(g)
    if legalize:
        _legalize_syncs(nc)
    return nc


def _priors_phase(tc, xT, Wr, pair, Ls, s0p, pools):
    nc = tc.nc
    mm_dt = xT.dtype
    # one psum drain per 32 n = 8 t-blocks = 2 DMA chunks
    for drain in range(N // 32):
        psum_t = pools["pr_psum"].tile([128, SLOTS, 64], FP32, tag="prpsum")
        for sub in range(2):
            chunk = drain * 2 + sub
            t0 = chunk * T_CHUNK
            x_tile = pools["xw"].tile([I, T_CHUNK, 4, B], mm_dt, tag="x")
            w_tile = pools["xw"].tile([I, T_CHUNK, 4, 64], mm_dt, tag="w")
            nc.sync.dma_start(
                out=x_tile[:],
                in_=xT[t0:t0 + T_CHUNK].rearrange("t p g b -> p t g b"))
            nc.sync.dma_start(
                out=w_tile[:],
                in_=Wr[pair, t0:t0 + T_CHUNK].rearrange("t p g f -> p t g f"))
            for tt in range(T_CHUNK):
                for g in range(4):
                    n_local = (sub * T_CHUNK + tt) * 4 + g
                    slot = (n_local // 2) % SLOTS
                    par = g & 1            # n parity = g parity (n = 4t+g)
                    nc.tensor.matmul(
                        psum_t[par * 64:(par + 1) * 64, slot, :],
                        x_tile[:, tt, g, :],
                        w_tile[:, tt, g, :],
                        start=True, stop=True,
                        tile_position=(0, par * 64))
        # drains on ScalarE (o-major into per-class L), it0 partial on DVE
        for cc in range(2):
            lview = Ls[cc][:, :, drain * SLOTS:(drain + 1) * SLOTS]
            nc.scalar.copy(lview.rearrange("p o s -> p s o"),
                           psum_t[:, :, cc * O:(cc + 1) * O])
        part = pools["small"].tile([128, 2, O], FP32, tag="s0part")
        nc.vector.tensor_reduce(
            part[:], psum_t[:].rearrange("p s (c o) -> p c o s", c=2),
            axis=mybir.AxisListType.X, op=ALU.add)
        if drain == 0:
            nc.vector.tensor_copy(s0p[:], part[:])
        else:
            nc.vector.tensor_add(s0p[:], s0p[:], part[:])


def _route_class(tc, Lc, sel_t, cc, pair, s0p, out4, pools):
    nc = tc.nc
    scratch, small, fold_psum = pools["scratch"], pools["small"], pools["fold_psum"]
    ctag = f"c{cc}"

    junk = scratch.tile([128, NR], FP32, tag=f"junk{ctag}")
    e_t = scratch.tile([128, NR], FP32, tag=f"e{ctag}")
    lG = scratch.tile([128, NR], FP32, tag=f"lG{ctag}")
    gtmp = scratch.tile([128, NR], FP32, tag=f"gtmp{ctag}")
    dbufs = [scratch.tile([128, NR], FP32, tag=f"ld{i}{ctag}",
                          name=f"ld{i}{ctag}{pair}")
             for i in range(2)]
    # sp holds the 32 weighted o-sums plus the softmax normalizer Z in col 32
    sp = small.tile([128, O + 1], FP32, tag=f"sp{ctag}")

    def fold(src_ap, ncols, tag, scale=None):
        """[128=(par,b), ncols] -> [128, ncols] (parity-summed, b-replicated
        across both partition halves) via sel2 matmul; lands in SBUF."""
        ps = fold_psum.tile([128, 64], FP32, tag="foldps")
        nc.tensor.matmul(ps[:, :ncols], sel_t[:], src_ap,
                         start=True, stop=True)
        dst = small.tile([128, ncols], FP32, tag=tag + ctag)
        if scale is None:
            nc.scalar.copy(dst[:], ps[:, :ncols])
        else:
            nc.scalar.activation(dst[:], ps[:, :ncols], AF.Copy, scale=scale)
        return dst

    def s_accumulate(weight):
        """sp[:, o] = sum_nr Lc[:, o, :] * weight[:, :]  (all DVE)."""
        for o in range(O):
            nc.vector.scalar_tensor_tensor(
                out=junk[:], in0=Lc[:, o, :], scalar=0.0, in1=weight[:],
                op0=ALU.bypass, op1=ALU.mult, accum_out=sp[:, o:o + 1])

    def delta_accumulate(outs, prev):
        """Return tile holding (prev-logits or 0) + sum_o Lc[:,o,:]*outs[:,o].
        DVE chains o<DVE_O (seeded with prev), GpSimd chains the rest."""
        cur = prev
        for o in range(DVE_O):
            dst = dbufs[0] if cur is not dbufs[0] else dbufs[1]
            if cur is None:
                nc.vector.scalar_tensor_tensor(
                    out=dst[:], in0=Lc[:, o, :], scalar=outs[:, o:o + 1],
                    in1=Lc[:, o, :], op0=ALU.mult, op1=ALU.bypass)
            else:
                nc.vector.scalar_tensor_tensor(
                    out=dst[:], in0=Lc[:, o, :], scalar=outs[:, o:o + 1],
                    in1=cur[:], op0=ALU.mult, op1=ALU.add)
            cur = dst
        first = True
        for o in range(DVE_O, O):
            # gpsimd tensor_scalar w/ AP scalar measured 8.7us -- use the
            # broadcast multiply (1.7us) then accumulate-add instead.
            if first:
                nc.gpsimd.tensor_mul(lG[:], Lc[:, o, :],
                                     outs[:, o:o + 1].to_broadcast((128, NR)))
                first = False
            else:
                nc.gpsimd.tensor_mul(gtmp[:], Lc[:, o, :],
                                     outs[:, o:o + 1].to_broadcast((128, NR)))
                nc.gpsimd.tensor_add(lG[:], lG[:], gtmp[:])
        nc.vector.tensor_add(cur[:], cur[:], lG[:])
        return cur

    def squash(s_sb):
        """outputs = s * sqrt(sq)/(1+sq), all [128, O] per-partition."""
        sq = small.tile([128, 1], FP32, tag="sq" + ctag)
        nc.vector.scalar_tensor_tensor(
            out=junk[:, :O], in0=s_sb[:], scalar=0.0, in1=s_sb[:],
            op0=ALU.bypass, op1=ALU.mult, accum_out=sq[:])
        lnq = small.tile([128, 1], FP32, tag="lnq" + ctag)
        nc.scalar.activation(lnq[:], sq[:], AF.Ln)
        r = small.tile([128, 1], FP32, tag="r" + ctag)
        nc.scalar.activation(r[:], lnq[:], AF.Exp, scale=0.5)  # sqrt(sq)
        q1 = small.tile([128, 1], FP32, tag="q1" + ctag)
        nc.vector.tensor_scalar_add(q1[:], sq[:], 1.0)
        iq = small.tile([128, 1], FP32, tag="iq" + ctag)
        nc.vector.reciprocal(iq[:], q1[:])
        f = small.tile([128, 1], FP32, tag="f" + ctag)
        nc.vector.tensor_mul(f[:], r[:], iq[:])
        outs = small.tile([128, O], FP32, tag="outs" + ctag)
        nc.vector.tensor_scalar(out=outs[:], in0=s_sb[:], scalar1=f[:],
                                scalar2=None, op0=ALU.mult)
        return outs

    # iteration 0: s0 = mean_n p (already accumulated from psum partials)
    s_sb = fold(s0p[:, cc], O, "s_sb", scale=INV_N)
    outs = squash(s_sb)
    yield
    logits = delta_accumulate(outs, None)
    yield

    for it in (1, 2):
        nc.scalar.activation(e_t[:], logits[:], AF.Exp,
                             accum_out=sp[:, O:O + 1])
        s_accumulate(e_t)
        yield
        sraw = fold(sp[:], O + 1, "s_sb")       # cols 0..31 = s, col 32 = Z
        iz = small.tile([128, 1], FP32, tag="iz" + ctag)
        nc.vector.reciprocal(iz[:], sraw[:, O:O + 1])
        s_sb = small.tile([128, O], FP32, tag="s_n" + ctag)
        nc.vector.tensor_scalar(out=s_sb[:], in0=sraw[:, :O], scalar1=iz[:],
                                scalar2=None, op0=ALU.mult)
        outs = squash(s_sb)
        yield
        if it == 1:
            logits = delta_accumulate(outs, logits)
            yield
        else:
            nc.sync.dma_start(out=out4[pair * 2 + cc], in_=outs[:B, :])


# ---------------------------------------------------------------------------
# host-side entry point
# ---------------------------------------------------------------------------

_COMPILED = {}


def _prep_host_inputs(x, route_weights):
    mm_np = np.float16 if USE_FP16 else np.float32
    x = np.ascontiguousarray(x, dtype=np.float32)
    W = np.ascontiguousarray(route_weights, dtype=np.float32)
    # xT[t, i, g, b] = x[b, 4t+g, i]
    xT = np.ascontiguousarray(
        x.reshape(B, T, 4, I).transpose(1, 3, 2, 0).astype(mm_np))
    sel2 = np.tile(np.eye(B, dtype=np.float32), (2, 2)).astype(np.float32)
    sel2 = np.ascontiguousarray(sel2)

    in_maps = []
    for k in range(N_CORES):
        Wk = W[k * CLASSES_PER_CORE:(k + 1) * CLASSES_PER_CORE]
        # Wr[pair, t, i, g, (cc,o)] = Wk[2*pair+cc, 4t+g, i, o]
        Wr = Wk.reshape(PAIRS, 2, T, 4, I, O).transpose(0, 2, 4, 3, 1, 5)
        Wr = np.ascontiguousarray(
            Wr.reshape(PAIRS, T, I, 4, 64).astype(mm_np))
        in_maps.append({"xT": xT, "Wr": Wr, "sel2": sel2})
    return in_maps


def _get_compiled():
    if "nc" not in _COMPILED:
        nc = bass.Bass("TRN2", target_bir_lowering=False, debug=False,
                       enable_asserts=False, num_devices=N_CORES)
        _build_program(nc)
        _COMPILED["nc"] = nc
    return _COMPILED["nc"]


def kernel(x, route_weights, **run_kwargs):
    in_maps = _prep_host_inputs(x, route_weights)
    nc = _get_compiled()
    res = bass_utils.run_bass_kernel_spmd(
        nc, in_maps, core_ids=list(range(N_CORES)), **run_kwargs)
    full = np.concatenate([r["out4"] for r in res.results], axis=0)
    out = full[:, :, None, None, :].astype(np.float32)
    if run_kwargs:
        kernel.last_results = res
    return out


if __name__ == "__main__":
    rng = np.random.default_rng(0)
    xs = rng.standard_normal((B, N, I), dtype=np.float32)
    ws = rng.standard_normal((C, N, I, O), dtype=np.float32)
    print(kernel(xs, ws).shape)
